# revision 2
# baseline (speedup 1.0000x reference)
"""Trainium2 Bass kernel for nn_EnergyGatedDelta.

Math
----
The encoder is pointwise per token and the vocabulary is only V=64, so
hs[b,l] = HS[seq[b,l]] for a 64x64 table HS, and likewise k = KT[c],
v = VT[c], q = QT[c].  With normalized keys KN[c] and the Gram matrix
G = KN @ KN.T, the delta-rule state M collapses to the per-class
residual table R[c] = v_c - M k_c (shape [64, 64] per batch element):

  per step with class c:  w = R[c];  fire iff |w|^2 > (0.4 |v_c|)^2
  if fire:  R[:, :] -= outer(G[:, c], w)        (G[c,c] = 1)

and the final read  M q = sum_c S[c] KQ[c, c_last]  can be streamed:
read = sum_fired w_t * KQ[c_t, c_last].  We store -read as a 65th row
of R whose "G" column is KQ[c_t, c_last], so it updates for free.

Scaling trick: rows are stored scaled by 1/s_c with s_c = 0.4|v_c|, and
G-hat[v,c] = G[v,c] * s_v / s_c, which makes the fire test simply
|w'|^2 > 1.0 (no per-class threshold lookup in the loop).

Layout per core (B_loc = 32 batch rows):
  4 "sets" of 8 batch rows; partitions = (8 b, 16 h-groups); free dims
  (65 classes, 4 h).  gpsimd indirect_copy gathers use one index per
  16-partition group == per batch row.
"""

import os
import sys
import threading

import numpy as np

sys.path.insert(0, os.path.dirname(os.path.abspath(__file__)))

import concourse.bass as bass
import concourse.mybir as mybir
import concourse.tile as tile
from concourse.bass_utils import run_bass_kernel_spmd


# ---------------------------------------------------------------------------
# Walrus workaround (inlined): this walrus build rejects instructions
# carrying more than one sync wait ("Too many sync wait commands").  After
# Tile finishes, move excess waits onto same-engine NoOps spliced before
# the overloaded instruction (same engine + earlier program order == same
# semantics).
# ---------------------------------------------------------------------------
from concourse.vector_clock import ScopedClock as _ScopedClock

_MWF_LIMIT = 1
_mwf_ctr = [0]


def _fix_multiwait(nc):
    for fn in nc.m.functions:
        for bb in fn.blocks:
            insts = bb.instructions
            i = 0
            while i < len(insts):
                inst = insts[i]
                si = inst.sync_info
                waits = list(si.on_wait) if si is not None and si.on_wait else []
                if len(waits) > _MWF_LIMIT:
                    si.on_wait = waits[:_MWF_LIMIT]
                    extra = waits[_MWF_LIMIT:]
                    pos = i
                    for j in range(0, len(extra), _MWF_LIMIT):
                        _mwf_ctr[0] += 1
                        nop = mybir.InstNoOp(
                            name=f"I-mwfix-{_mwf_ctr[0]}", ins=[], outs=[]
                        )
                        nop.engine = inst.engine
                        nop.sync_info = mybir.SyncInfo(
                            on_wait=extra[j : j + _MWF_LIMIT], on_update=[]
                        )
                        insts.insert(pos, nop)
                        pos += 1
                        i += 1
                i += 1
            bb.instructions = insts


def _patched_drain_and_barrier(self, tick_clock, wait_clock):
    nop_inst = self.nc.sync.nop(nofuse=True)
    wait_clock.add_sem_waits(
        nop_inst.ins, _ScopedClock({None: tick_clock.global_clock})
    )
    self.nc.sync.drain()
    self.nc.all_engine_barrier()
    assert self.sems is not None
    popped = self.nc._tile_sem_poison_stack.pop()
    assert popped is self._sem_poison
    self.nc.clear_and_free_semaphores(list(self.sems.allocated().values()))
    self.nc.all_engine_barrier()
    _fix_multiwait(self.nc)


tile.TileContext._drain_and_barrier = _patched_drain_and_barrier

F32 = mybir.dt.float32
I32 = mybir.dt.int32
U16 = mybir.dt.uint16
OP = mybir.AluOpType
AF = mybir.ActivationFunctionType

B = 256
L = 4096
H = 64
V = 64
NCORES = 8
BLOC = B // NCORES          # 32
NSETS = 4                   # 4 sets x 8 batch rows
NSTEPS = L - 1              # 4095
UNROLL = 32
NWIN = (L - 1) // UNROLL    # 127 full windows; tail handled per-step
LN_EPS = 1e-5
NORM_EPS = 1e-12

_cache = threading.Lock()
_built = {}


def _build():
    nc = bass.Bass()
    # Unused input whose shape salts the MLIR fingerprint: the axon
    # terminal caches executables by module hash and would otherwise
    # serve a stale NEFF across kernel revisions.
    import random

    nonce_n = random.randint(2, 509)
    nc.dram_tensor("nonce", [1, nonce_n], F32, kind="ExternalInput")

    # ---------------- DRAM I/O ----------------
    seq_d = nc.dram_tensor("seq", [BLOC, L], I32, kind="ExternalInput")
    emb_d = nc.dram_tensor("embed", [V, H], F32, kind="ExternalInput")
    w1_d = nc.dram_tensor("W1", [H, 2 * H], F32, kind="ExternalInput")
    b1_d = nc.dram_tensor("b1", [1, 2 * H], F32, kind="ExternalInput")
    w2_d = nc.dram_tensor("W2", [2 * H, H], F32, kind="ExternalInput")
    b2_d = nc.dram_tensor("b2", [1, H], F32, kind="ExternalInput")
    lng_d = nc.dram_tensor("ln_g", [1, H], F32, kind="ExternalInput")
    lnb_d = nc.dram_tensor("ln_b", [1, H], F32, kind="ExternalInput")
    wk_d = nc.dram_tensor("Wk", [H, H], F32, kind="ExternalInput")
    wv_d = nc.dram_tensor("Wv", [H, H], F32, kind="ExternalInput")
    wq_d = nc.dram_tensor("Wq", [H, H], F32, kind="ExternalInput")
    wrp_d = nc.dram_tensor("Wrp", [H, H], F32, kind="ExternalInput")
    brp_d = nc.dram_tensor("brp", [1, H], F32, kind="ExternalInput")
    wout_d = nc.dram_tensor("Wout", [H, V], F32, kind="ExternalInput")
    bout_d = nc.dram_tensor("bout", [1, V], F32, kind="ExternalInput")
    out_d = nc.dram_tensor("out", [BLOC, V], F32, kind="ExternalOutput")

    with tile.TileContext(nc) as tc:
        with (
            tc.tile_pool(name="state", bufs=1) as st,
            tc.tile_pool(name="scratch", bufs=1) as sc,
            tc.tile_pool(name="loop", bufs=3) as lp,
            tc.tile_pool(name="gbuf", bufs=1) as gbp,
            tc.tile_pool(name="psum", bufs=3, space="PSUM") as pp,
            tc.tile_pool(name="lpsum", bufs=4, space="PSUM") as lpp,
            tc.tile_pool(name="dram", bufs=1, space="DRAM") as dp,
        ):
            # ---------------- constants ----------------
            ident = st.tile([128, 128], F32, tag="ident")
            from concourse.masks import make_identity

            make_identity(nc, ident[:])

            # GRP[p, q] = 1.0 if p//16 == q//16  (group-sum + replicate)
            # built as AT.T @ AT with AT[g, q] = (q//16 == g)
            at = sc.tile([8, 128], F32, tag="at")
            nc.gpsimd.memset(at[:], 1.0)
            nc.gpsimd.affine_select(
                out=at[:], in_=at[:], pattern=[[1, 128]],
                compare_op=OP.is_ge, fill=0.0, base=0, channel_multiplier=-16,
            )
            nc.gpsimd.affine_select(
                out=at[:], in_=at[:], pattern=[[-1, 128]],
                compare_op=OP.is_ge, fill=0.0, base=15, channel_multiplier=16,
            )
            grp_ps = pp.tile([128, 128], F32, tag="pre", space="PSUM")
            nc.tensor.matmul(grp_ps[:], at[:], at[:], start=True, stop=True)
            grp = st.tile([128, 128], F32, tag="grp")
            nc.vector.tensor_copy(grp[:], grp_ps[:])

            ones1x64 = st.tile([1, 64], F32, tag="o64")
            ones1x128 = st.tile([1, 128], F32, tag="o128")
            ones1x32 = st.tile([1, 32], F32, tag="o32")
            nc.vector.memset(ones1x64[:], 1.0)
            nc.vector.memset(ones1x128[:], 1.0)
            nc.vector.memset(ones1x32[:], 1.0)

            # ---------------- load weights ----------------
            emb = sc.tile([V, H], F32, tag="emb")
            w1 = sc.tile([H, 2 * H], F32, tag="w1")
            w2 = sc.tile([2 * H, H], F32, tag="w2")
            wk = sc.tile([H, H], F32, tag="wk")
            wv = sc.tile([H, H], F32, tag="wv")
            wq = sc.tile([H, H], F32, tag="wq")
            wrpn = st.tile([H, H], F32, tag="wrpn")
            wout = st.tile([H, V], F32, tag="wout")
            b1t = sc.tile([128, 1], F32, tag="b1t")
            b2r = sc.tile([1, H], F32, tag="b2r")
            lngr = sc.tile([1, H], F32, tag="lngr")
            lnbr = sc.tile([1, H], F32, tag="lnbr")
            brpr = st.tile([1, H], F32, tag="brpr")
            boutr = st.tile([1, V], F32, tag="boutr")
            nc.sync.dma_start(emb[:], emb_d[:])
            nc.sync.dma_start(w1[:], w1_d[:])
            nc.sync.dma_start(w2[:], w2_d[:])
            nc.sync.dma_start(wk[:], wk_d[:])
            nc.sync.dma_start(wv[:], wv_d[:])
            nc.sync.dma_start(wq[:], wq_d[:])
            nc.sync.dma_start(wrpn[:], wrp_d[:])
            nc.sync.dma_start(wout[:], wout_d[:])
            # b1 as [128,1] via strided DMA (transpose of a vector)
            nc.sync.dma_start(b1t[:], b1_d[0, :].unsqueeze(1))
            nc.sync.dma_start(b2r[:], b2_d[:])
            nc.sync.dma_start(lngr[:], lng_d[:])
            nc.sync.dma_start(lnbr[:], lnb_d[:])
            nc.sync.dma_start(brpr[:], brp_d[:])
            nc.sync.dma_start(boutr[:], bout_d[:])
            # negate Wrp (final read is stored negated)
            nc.vector.tensor_scalar_mul(wrpn[:], wrpn[:], -1.0)

            # ---------------- encoder table ----------------
            # embT
            embT_ps = pp.tile([H, V], F32, tag="pre", space="PSUM")
            nc.tensor.transpose(embT_ps[:], emb[:], ident[0:V, 0:V])
            embT = sc.tile([H, V], F32, tag="embT")
            nc.scalar.activation(embT[:], embT_ps[:], AF.Copy)
            # h1T = relu(W1.T @ e.T + b1)   [128, 64]
            h1_ps = pp.tile([2 * H, V], F32, tag="pre", space="PSUM")
            nc.tensor.matmul(h1_ps[:], w1[:], embT[:], start=True, stop=True)
            h1t = sc.tile([2 * H, V], F32, tag="h1t")
            nc.scalar.activation(h1t[:], h1_ps[:], AF.Relu, bias=b1t[:], scale=1.0)
            # x = e + h1 @ W2 + b2     [64v, 64h]
            x_ps = pp.tile([V, H], F32, tag="pre", space="PSUM")
            nc.tensor.matmul(x_ps[:], h1t[:], w2[:], start=True, stop=False)
            nc.tensor.matmul(x_ps[:], ident[0:V, 0:V], emb[:], start=False, stop=False)
            nc.tensor.matmul(x_ps[:], ones1x64[:], b2r[:], start=False, stop=True)
            # layernorm
            mu = sc.tile([V, 1], F32, tag="mu")
            nc.vector.tensor_reduce(mu[:], x_ps[:], mybir.AxisListType.X, OP.add)
            nc.vector.tensor_scalar_mul(mu[:], mu[:], 1.0 / H)
            xc = sc.tile([V, H], F32, tag="xc")
            nc.vector.tensor_scalar(xc[:], x_ps[:], mu[:], None, OP.subtract)
            junkA = sc.tile([V, H], F32, tag="junkA")
            var_s = sc.tile([V, 1], F32, tag="var_s")
            nc.vector.scalar_tensor_tensor(
                out=junkA[:], in0=xc[:], scalar=1.0, in1=xc[:],
                op0=OP.mult, op1=OP.mult, accum_out=var_s[:],
            )
            epst = sc.tile([V, 1], F32, tag="epst")
            nc.vector.memset(epst[:], LN_EPS)
            sig = sc.tile([V, 1], F32, tag="sig")
            nc.scalar.activation(sig[:], var_s[:], AF.Sqrt, bias=epst[:], scale=1.0 / H)
            rstd = sc.tile([V, 1], F32, tag="rstd")
            nc.vector.reciprocal(rstd[:], sig[:])
            lngB_ps = pp.tile([V, H], F32, tag="pre", space="PSUM")
            nc.tensor.matmul(lngB_ps[:], ones1x64[:], lngr[:], start=True, stop=True)
            lnbB_ps = pp.tile([V, H], F32, tag="pre", space="PSUM")
            nc.tensor.matmul(lnbB_ps[:], ones1x64[:], lnbr[:], start=True, stop=True)
            hs = sc.tile([V, H], F32, tag="hs")
            nc.vector.scalar_tensor_tensor(
                out=hs[:], in0=xc[:], scalar=rstd[:], in1=lngB_ps[:],
                op0=OP.mult, op1=OP.mult,
            )
            nc.vector.tensor_tensor(hs[:], hs[:], lnbB_ps[:], OP.add)
            # hsT
            hsT_ps = pp.tile([H, V], F32, tag="pre", space="PSUM")
            nc.tensor.transpose(hsT_ps[:], hs[:], ident[0:V, 0:V])
            hsT = sc.tile([H, V], F32, tag="hsT")
            nc.scalar.activation(hsT[:], hsT_ps[:], AF.Copy)

            # K/V/Q tables  [64v(class), 64h]
            kt_ps = pp.tile([V, H], F32, tag="pre", space="PSUM")
            nc.tensor.matmul(kt_ps[:], hsT[:], wk[:], start=True, stop=True)
            kt = sc.tile([V, H], F32, tag="kt")
            nc.scalar.activation(kt[:], kt_ps[:], AF.Copy)
            vt_ps = pp.tile([V, H], F32, tag="pre", space="PSUM")
            nc.tensor.matmul(vt_ps[:], hsT[:], wv[:], start=True, stop=True)
            vt = sc.tile([V, H], F32, tag="vt")
            nc.scalar.activation(vt[:], vt_ps[:], AF.Copy)
            qt_ps = pp.tile([V, H], F32, tag="pre", space="PSUM")
            nc.tensor.matmul(qt_ps[:], hsT[:], wq[:], start=True, stop=True)
            qt = sc.tile([V, H], F32, tag="qt")
            nc.scalar.activation(qt[:], qt_ps[:], AF.Copy)

            # normalized keys
            junkB = sc.tile([V, H], F32, tag="junkB")
            kn2 = sc.tile([V, 1], F32, tag="kn2")
            nc.vector.scalar_tensor_tensor(
                out=junkB[:], in0=kt[:], scalar=1.0, in1=kt[:],
                op0=OP.mult, op1=OP.mult, accum_out=kn2[:],
            )
            knrm = sc.tile([V, 1], F32, tag="knrm")
            nc.scalar.activation(knrm[:], kn2[:], AF.Sqrt)
            nc.vector.tensor_scalar_max(knrm[:], knrm[:], NORM_EPS)
            rkn = sc.tile([V, 1], F32, tag="rkn")
            nc.vector.reciprocal(rkn[:], knrm[:])
            kn = sc.tile([V, H], F32, tag="kn")
            nc.vector.tensor_scalar(kn[:], kt[:], rkn[:], None, OP.mult)

            # G = KN @ KN.T ; s_c = 0.4*|v_c| ; Ghat = G * s_v / s_c
            knT_ps = pp.tile([H, V], F32, tag="pre", space="PSUM")
            nc.tensor.transpose(knT_ps[:], kn[:], ident[0:V, 0:V])
            knT = sc.tile([H, V], F32, tag="knT")
            nc.scalar.activation(knT[:], knT_ps[:], AF.Copy)
            g_ps = pp.tile([V, V], F32, tag="pre", space="PSUM")
            nc.tensor.matmul(g_ps[:], knT[:], knT[:], start=True, stop=True)
            g_sb = sc.tile([V, V], F32, tag="g_sb")
            nc.scalar.activation(g_sb[:], g_ps[:], AF.Copy)

            junkC = sc.tile([V, H], F32, tag="junkC")
            vn2 = sc.tile([V, 1], F32, tag="vn2")
            nc.vector.scalar_tensor_tensor(
                out=junkC[:], in0=vt[:], scalar=1.0, in1=vt[:],
                op0=OP.mult, op1=OP.mult, accum_out=vn2[:],
            )
            s_col = sc.tile([V, 1], F32, tag="s_col")
            nc.scalar.activation(s_col[:], vn2[:], AF.Sqrt, scale=0.16)
            nc.vector.tensor_scalar_max(s_col[:], s_col[:], NORM_EPS)
            rs_col = sc.tile([V, 1], F32, tag="rs_col")
            nc.vector.reciprocal(rs_col[:], s_col[:])

            # rows of s and 1/s broadcast
            srow_ps = pp.tile([1, V], F32, tag="pre", space="PSUM")
            nc.tensor.transpose(srow_ps[:], s_col[:], ident[0:V, 0:V][:, 0:V])
            srow = sc.tile([1, V], F32, tag="srow")
            nc.vector.tensor_copy(srow[:], srow_ps[:])
            rsrow_ps = pp.tile([1, V], F32, tag="pre", space="PSUM")
            nc.tensor.transpose(rsrow_ps[:], rs_col[:], ident[0:V, 0:V][:, 0:V])
            rsrow = sc.tile([1, V], F32, tag="rsrow")
            nc.vector.tensor_copy(rsrow[:], rsrow_ps[:])
            rsB_ps = pp.tile([V, V], F32, tag="pre", space="PSUM")
            nc.tensor.matmul(rsB_ps[:], ones1x64[:], rsrow[:], start=True, stop=True)
            sB128_ps = pp.tile([128, V], F32, tag="pre", space="PSUM")
            nc.tensor.matmul(sB128_ps[:], ones1x128[:], srow[:], start=True, stop=True)
            sB128 = st.tile([128, V], F32, tag="sB128")
            nc.vector.tensor_copy(sB128[:], sB128_ps[:])

            # Gsc: cols 0-63 = G, col 64 = kappa (per set), col 65 = TH2
            th2v = sc.tile([V, 1], F32, tag="th2v")
            nc.vector.tensor_scalar_mul(th2v[:], vn2[:], 0.16)
            gsc = sc.tile([V, 68], F32, tag="gsc")
            nc.vector.memset(gsc[:, 64:68], 0.0)
            nc.vector.tensor_copy(gsc[:, 0:64], g_sb[:])
            nc.vector.tensor_copy(gsc[:, 65:66], th2v[:])
            gsc_d = dp.tile([V, 68], F32, tag="gsc_d")
            nc.sync.dma_start(gsc_d[:], gsc[:])

            # KQT[c, c'] = sum_h QT[c,h] KN[c',h]
            qtT_ps = pp.tile([H, V], F32, tag="pre", space="PSUM")
            nc.tensor.transpose(qtT_ps[:], qt[:], ident[0:V, 0:V])
            qtT = sc.tile([H, V], F32, tag="qtT")
            nc.scalar.activation(qtT[:], qtT_ps[:], AF.Copy)
            kqt_ps = pp.tile([V, V], F32, tag="pre", space="PSUM")
            nc.tensor.matmul(kqt_ps[:], qtT[:], knT[:], start=True, stop=True)
            kqt = sc.tile([V, V], F32, tag="kqt")
            nc.scalar.activation(kqt[:], kqt_ps[:], AF.Copy)


            vts_d = dp.tile([V, H], F32, tag="vts_d")
            nc.sync.dma_start(vts_d[:], vt[:])

            # POFF[p, c2] = 4*(p % 16) + 64*c2   (wrapped chunk offsets)
            pidx = sc.tile([128, 1], U16, tag="pidx")
            nc.gpsimd.iota(pidx[:], [[0, 1]], channel_multiplier=1)
            pm16 = sc.tile([128, 1], U16, tag="pm16")
            nc.vector.tensor_scalar(pm16[:], pidx[:], 15, None, OP.bitwise_and)
            nc.vector.tensor_scalar(pm16[:], pm16[:], 4, None, OP.mult)
            c2base = sc.tile([128, 2], U16, tag="c2base")
            nc.gpsimd.iota(c2base[:], [[64, 2]], channel_multiplier=0)
            poff = st.tile([128, 2], U16, tag="poff")
            nc.vector.tensor_tensor(
                poff[:], c2base[:], pm16[:].to_broadcast([128, 2]), OP.add
            )

            # ---------------- per-set state ----------------
            r_sets = []
            gaug_sets = []
            seqf_sets = []
            gidxw_sets = []
            for s in range(NSETS):
                r_t = st.tile([128, 68, 4], F32, tag=f"r{s}")
                gaug = st.tile([128, V, 68], F32, tag=f"gaug{s}")
                seqf = st.tile([128, L], U16, tag=f"seqf{s}")

                r_sets.append(r_t)
                gaug_sets.append(gaug)
                seqf_sets.append(seqf)

                # R init: partition (b, a) rows c get vts[c, 4a:4a+4]
                for a in range(16):
                    nc.sync.dma_start(
                        r_t[a : 128 : 16, 0:64, :],
                        vts_d[:, 4 * a : 4 * a + 4]
                        .unsqueeze(0)
                        .to_broadcast([8, 64, 4]),
                    )
                nc.vector.memset(r_t[:, 64:68, :], 0.0)

                # Gaug rows from DRAM broadcast
                nc.sync.dma_start(
                    gaug[:].rearrange("p v c -> p (v c)"),
                    gsc_d[:]
                    .rearrange("v c -> (v c)")
                    .unsqueeze(0)
                    .to_broadcast([128, 68 * V]),
                )

                # seq (low u16 halves) replicated onto every partition of
                # its 16-partition group, via bitcast strided DMA
                for a in range(16):
                    nc.sync.dma_start(
                        seqf[a : 128 : 16, :],
                        seq_d[8 * s : 8 * s + 8, :]
                        .bitcast(U16)[:, 0 : 2 * L : 2],
                    )

                # kappa column: KQT[c_last[b], :] * s_v
                clast = sc.tile([128, 1], I32, tag="clast")
                for a in range(16):
                    nc.sync.dma_start(
                        clast[a : 128 : 16, :],
                        seq_d[8 * s : 8 * s + 8, L - 1 : L],
                    )
                # kappa[p, v] = KQT[clast_p, v] via one-hot matmul
                clf = sc.tile([128, 1], F32, tag="clf")
                nc.vector.tensor_copy(clf[:], clast[:])
                clrow_ps = pp.tile([1, 128], F32, tag="pre", space="PSUM")
                nc.tensor.transpose(clrow_ps[:], clf[:], ident[:, :])
                clrow = sc.tile([1, 128], F32, tag="clrow")
                nc.vector.tensor_copy(clrow[:], clrow_ps[:])
                clB_ps = pp.tile([V, 128], F32, tag="pre", space="PSUM")
                nc.tensor.matmul(clB_ps[:], ones1x64[:], clrow[:], start=True, stop=True)
                iotac = sc.tile([V, 1], mybir.dt.int16, tag="iotac")
                nc.gpsimd.iota(iotac[:], [[0, 1]], channel_multiplier=1)
                iotacf = sc.tile([V, 1], F32, tag="iotacf")
                nc.vector.tensor_copy(iotacf[:], iotac[:])
                eh = sc.tile([V, 128], F32, tag="eh")
                nc.vector.tensor_scalar(eh[:], clB_ps[:], iotacf[:], None, OP.is_equal)
                kap_ps = pp.tile([128, V], F32, tag="pre", space="PSUM")
                nc.tensor.matmul(kap_ps[:], eh[:], kqt[:], start=True, stop=True)
                nc.vector.tensor_copy(gaug[:, :, 64], kap_ps[:])

                # index tiles: only group-leader partitions matter


            # ---------------- main scan ----------------
            upd_engine = [nc.vector, nc.vector, nc.vector, nc.vector]

            def step(t, wst=None, gst=None, k=0, growB=None):
                for s in range(NSETS):
                    r_t = r_sets[s]
                    w4 = lp.tile([128, 1, 4], F32, tag=f"w4_{s}")
                    widx_ap = wst[s][:, k, 0:1]
                    nc.gpsimd.indirect_copy(
                        w4[:], r_t[:], widx_ap,
                        i_know_ap_gather_is_preferred=True,
                    )
                    if growB is not None:
                        grow = growB[s][:, 17 * k : 17 * k + 17, :]
                    else:
                        grow_t = lp.tile([128, 17, 4], F32, tag=f"grow_{s}")
                        nc.gpsimd.indirect_copy(
                            grow_t[:],
                            gaug_sets[s][:].rearrange(
                                "p v (j h) -> p (v j) h", h=4
                            ),
                            gst[s][:, k, :],
                            i_know_ap_gather_is_preferred=True,
                        )
                        grow = grow_t[:]
                    j4 = lp.tile([128, 1, 4], F32, tag=f"j4_{s}")
                    n2p = lp.tile([128, 1], F32, tag=f"n2p_{s}")
                    nc.scalar.activation(
                        j4[:], w4[:], AF.Square, accum_out=n2p[:]
                    )
                    n2ps = lpp.tile([128, 1], F32, tag="n2", space="PSUM")
                    nc.tensor.matmul(n2ps[:], grp[:], n2p[:], start=True, stop=True)
                    gm = lp.tile([128, 1], F32, tag=f"gm_{s}")
                    nc.vector.tensor_tensor(
                        gm[:], n2ps[:], grow[:, 16, 1:2], OP.is_gt
                    )
                    tmp = lp.tile([128, 68, 4], F32, tag=f"tmp_{s}")
                    eng = upd_engine[s]
                    if eng is nc.vector:
                        eng.scalar_tensor_tensor(
                            out=tmp[:],
                            in0=w4[:, 0, :].unsqueeze(1).to_broadcast([128, 68, 4]),
                            scalar=gm[:],
                            in1=grow.rearrange("p j h -> p (j h)").unsqueeze(2).to_broadcast([128, 68, 4]),
                            op0=OP.mult,
                            op1=OP.mult,
                        )
                    else:
                        wg = lp.tile([128, 1, 4], F32, tag=f"wg_{s}")
                        nc.vector.tensor_scalar(wg[:], w4[:], gm[:], None, OP.mult)
                        eng.tensor_tensor(
                            tmp[:],
                            wg[:, 0, :].unsqueeze(1).to_broadcast([128, 68, 4]),
                            grow.rearrange("p j h -> p (j h)").unsqueeze(2).to_broadcast([128, 68, 4]),
                            OP.mult,
                        )
                    eng.tensor_tensor(r_t[:], r_t[:], tmp[:], OP.subtract)

            def make_stages(n, slicer):
                wst = []
                gst = []
                for s in range(NSETS):
                    w_stage = lp.tile([128, n, 2], U16, tag=f"wstg_{s}")
                    s68 = lp.tile([128, n], U16, tag=f"s68_{s}")
                    g_stage = lp.tile([128, n, 2], U16, tag=f"gstg_{s}")
                    nc.vector.tensor_scalar(
                        w_stage[:, :, 0], slicer(seqf_sets[s]), 4, None, OP.mult
                    )
                    nc.vector.tensor_scalar(
                        s68[:], slicer(seqf_sets[s]), 68, None, OP.mult
                    )
                    nc.vector.tensor_tensor(
                        g_stage[:],
                        s68[:].unsqueeze(2).to_broadcast([128, n, 2]),
                        poff[:].unsqueeze(1).to_broadcast([128, n, 2]),
                        OP.add,
                    )
                    wst.append(w_stage)
                    gst.append(g_stage)
                return wst, gst

            n_main = NWIN * UNROLL
            with tc.For_i(0, NWIN, 1) as wv:
                wst, gst = make_stages(
                    UNROLL, lambda tile_: tile_[:, bass.ds(wv * UNROLL, UNROLL)]
                )
                for k in range(UNROLL):
                    step(None, wst, gst, k)
            n_tail = NSTEPS - n_main
            if n_tail:
                wst, gst = make_stages(
                    n_tail, lambda tile_: tile_[:, n_main:NSTEPS]
                )
                for k in range(n_tail):
                    step(None, wst, gst, k)

            # ---------------- readout ----------------
            readN = sc.tile([BLOC, H], F32, tag="readN")
            for s in range(NSETS):
                for a in range(16):
                    nc.sync.dma_start(
                        readN[8 * s : 8 * s + 8, 4 * a : 4 * a + 4],
                        r_sets[s][a : 128 : 16, 64, :],
                    )
            readT_ps = pp.tile([H, BLOC], F32, tag="pre", space="PSUM")
            nc.tensor.transpose(readT_ps[:], readN[:], ident[0:BLOC, 0:BLOC])
            readT = sc.tile([H, BLOC], F32, tag="readT")
            nc.scalar.activation(readT[:], readT_ps[:], AF.Copy)
            o1_ps = pp.tile([BLOC, H], F32, tag="pre", space="PSUM")
            nc.tensor.matmul(o1_ps[:], readT[:], wrpn[:], start=True, stop=False)
            nc.tensor.matmul(o1_ps[:], ones1x32[:], brpr[:], start=False, stop=True)
            o1 = sc.tile([BLOC, H], F32, tag="o1")
            nc.scalar.activation(o1[:], o1_ps[:], AF.Copy)
            o1T_ps = pp.tile([H, BLOC], F32, tag="pre", space="PSUM")
            nc.tensor.transpose(o1T_ps[:], o1[:], ident[0:BLOC, 0:BLOC])
            o1T = sc.tile([H, BLOC], F32, tag="o1T")
            nc.scalar.activation(o1T[:], o1T_ps[:], AF.Copy)
            o2_ps = pp.tile([BLOC, V], F32, tag="pre", space="PSUM")
            nc.tensor.matmul(o2_ps[:], o1T[:], wout[:], start=True, stop=False)
            nc.tensor.matmul(o2_ps[:], ones1x32[:], boutr[:], start=False, stop=True)
            o2 = sc.tile([BLOC, V], F32, tag="o2")
            nc.scalar.activation(o2[:], o2_ps[:], AF.Copy)
            nc.sync.dma_start(out_d[:], o2[:])

    return nc


def _get_nc():
    with _cache:
        if "nc" not in _built:
            _built["nc"] = _build()
    return _built["nc"]


def _make_in_maps(inputs, nc=None):
    seq = np.ascontiguousarray(np.asarray(inputs["seq"], dtype=np.int32))
    assert seq.shape == (B, L)

    def row(name, n):
        return np.ascontiguousarray(
            np.asarray(inputs[name], dtype=np.float32).reshape(1, n)
        )

    common = {
        "embed": np.ascontiguousarray(np.asarray(inputs["embed"], np.float32)),
        "W1": np.ascontiguousarray(np.asarray(inputs["W1"], np.float32)),
        "b1": row("b1", 2 * H),
        "W2": np.ascontiguousarray(np.asarray(inputs["W2"], np.float32)),
        "b2": row("b2", H),
        "ln_g": row("ln_g", H),
        "ln_b": row("ln_b", H),
        "Wk": np.ascontiguousarray(np.asarray(inputs["Wk"], np.float32)),
        "Wv": np.ascontiguousarray(np.asarray(inputs["Wv"], np.float32)),
        "Wq": np.ascontiguousarray(np.asarray(inputs["Wq"], np.float32)),
        "Wrp": np.ascontiguousarray(np.asarray(inputs["Wrp"], np.float32)),
        "brp": row("brp", H),
        "Wout": np.ascontiguousarray(np.asarray(inputs["Wout"], np.float32)),
        "bout": row("bout", V),
    }
    i = np.arange(17 * 8)
    t_loc = (i // 17).astype(np.int64)
    j4 = (4 * (i % 17)).astype(np.uint16)
    nwin = NWIN * 4
    if nc is None:
        nc = _get_nc()
    nonce_shape = None
    for alloc in nc.m.functions[0].allocations:
        try:
            nm = alloc.memorylocations[0].name
        except Exception:
            continue
        if nm == "nonce":
            nonce_shape = tuple(alloc.tensor_shape)
    in_maps = []
    for c in range(NCORES):
        m = dict(common)
        m["seq"] = np.ascontiguousarray(seq[c * BLOC : (c + 1) * BLOC])
        if nonce_shape is not None:
            m["nonce"] = np.zeros(nonce_shape, np.float32)
        in_maps.append(m)
    return in_maps




# ---------------------------------------------------------------------------
# Cached PJRT runner: build the jitted shard_map executable once and reuse
# it across calls (run_bass_kernel_spmd re-traces and re-lowers every call,
# which costs ~300 ms of host overhead per invocation).
# ---------------------------------------------------------------------------
def _get_runner():
    with _cache:
        if "runner" in _built:
            return _built["runner"]
    nc = _get_nc()
    import jax
    from jax.sharding import Mesh, PartitionSpec
    from jax.experimental.shard_map import shard_map
    from concourse.bass2jax import (
        _bass_exec_p,
        install_neuronx_cc_hook,
        partition_id_tensor,
    )

    install_neuronx_cc_hook()
    partition_name = nc.partition_id_tensor.name if nc.partition_id_tensor else None
    in_names, out_names, out_avals, zero_shapes = [], [], [], []
    for alloc in nc.m.functions[0].allocations:
        if not isinstance(alloc, mybir.MemoryLocationSet):
            continue
        name = alloc.memorylocations[0].name
        if alloc.kind == "ExternalInput":
            if name != partition_name:
                in_names.append(name)
        elif alloc.kind == "ExternalOutput":
            shape = tuple(alloc.tensor_shape)
            dtype = mybir.dt.np(alloc.dtype)
            out_names.append(name)
            out_avals.append(jax.core.ShapedArray(shape, dtype))
            zero_shapes.append(((NCORES * shape[0], *shape[1:]), dtype))
    n_params = len(in_names)
    all_in = in_names + out_names + ([partition_name] if partition_name else [])

    def _body(*args):
        operands = list(args)
        if partition_name is not None:
            operands.append(partition_id_tensor())
        outs = _bass_exec_p.bind(
            *operands, out_avals=tuple(out_avals), in_names=tuple(all_in),
            out_names=tuple(out_names), lowering_input_output_aliases=(),
            sim_require_finite=True, sim_require_nnan=True, nc=nc,
        )
        return tuple(outs)

    devices = jax.devices()[:NCORES]
    mesh = Mesh(np.asarray(devices), ("core",))
    n_outs = len(out_names)
    sharded = jax.jit(
        shard_map(
            _body, mesh=mesh,
            in_specs=(PartitionSpec("core"),) * (n_params + n_outs),
            out_specs=(PartitionSpec("core"),) * n_outs,
            check_rep=False,
        ),
        donate_argnums=tuple(range(n_params, n_params + n_outs)),
        keep_unused=True,
    )

    def run(in_maps):
        concat_in = [
            np.concatenate([m[name] for m in in_maps], axis=0) for name in in_names
        ]
        zeros = [np.zeros(shp, dt) for shp, dt in zero_shapes]
        out = sharded(*concat_in, *zeros)
        res = [np.asarray(o) for o in out]
        return {
            name: res[i].reshape(NCORES, *out_avals[i].shape)
            for i, name in enumerate(out_names)
        }

    with _cache:
        _built["runner"] = run
    return run


def kernel(**inputs):
    run = _get_runner()
    in_maps = _make_in_maps(inputs)
    res = run(in_maps)
    out = res["out"].reshape(B, V)
    return out.astype(np.float32)


if __name__ == "__main__":
    rng = np.random.default_rng(0)
    ins = {
        "seq": rng.integers(0, V, (B, L)).astype(np.int32),
        "embed": rng.standard_normal((V, H), np.float32),
        "W1": (rng.standard_normal((H, 2 * H)) / 8).astype(np.float32),
        "b1": np.zeros(2 * H, np.float32),
        "W2": (rng.standard_normal((2 * H, H)) / 11.3).astype(np.float32),
        "b2": np.zeros(H, np.float32),
        "ln_g": np.ones(H, np.float32),
        "ln_b": np.zeros(H, np.float32),
        "Wk": (rng.standard_normal((H, H)) / 8).astype(np.float32),
        "Wv": (rng.standard_normal((H, H)) / 8).astype(np.float32),
        "Wq": (rng.standard_normal((H, H)) / 8).astype(np.float32),
        "Wrp": (rng.standard_normal((H, H)) / 8).astype(np.float32),
        "brp": np.zeros(H, np.float32),
        "Wout": (rng.standard_normal((H, V)) / 8).astype(np.float32),
        "bout": np.zeros(V, np.float32),
    }
    out = kernel(**ins)
    print("out", out.shape, out.dtype, float(np.abs(out).max()))

